# revision 11
# baseline (speedup 1.0000x reference)
"""AttnPool3D Trainium2 kernel.

Reference computation (B=2, C=128, D=48, H=96, W=96, N = D*H*W = 442368):
    logits = einsum('bcdhw,c->bdhw', feat, w_attn) + 2.0*clip(mask, 0, 1)
    w = softmax(logits.reshape(B, -1), axis=-1)
    out = einsum('bcn,bn->bc', feat.reshape(B, C, -1), w)

Sharding: 8 cores = (batch b in 0..1) x (spatial quarter q in 0..3).
Each core processes feat[b, :, q*Ns:(q+1)*Ns] (Ns = 110592) in ONE pass.

Numerics: feat is shipped as fp16 (fh). Per-element fp16 rounding noise is
independent of the softmax weights, so it averages out in the final weighted
sums (measured ~1e-5 L2 rel err end-to-end). The w vector, by contrast, is
fp16-split (w = wh + wl) because its rounding error correlates with the
pass-2 data and does NOT average out (dropping wl costs 2e-4 rel err).
The mask is fp16-split the same way (mh + ml rows, one K=2 matmul).
Softmax runs without a max pass: logits are bounded (~N(0,1.3)+[0,2]); a
constant bias -8 in the exp prevents overflow and cancels in v/s.

Per chunk of 2048 spatial columns (54 chunks):
    - DMA fh chunk [128, 2048] fp16 (C on partitions), mask rows [2, 2048]
    - PE (per 512-sub-chunk, accumulating into PSUM [128, 512]):
        X  = wh_rep^T @ fh       (broadcasts logits to all 128 partitions)
        X += wl_rep^T @ fh
        X += ones2^T @ [mh; ml]
    - ACT: Pb = exp(X - 8), accum_out -> s_chunk [128, 1]
    - DVE: stt junk = fh * Pb (fp32 ALU), accum_out -> v_chunk [128, 1]
Host combines: out[b, c] = sum_q v / sum_q s (fp64; the -8 bias cancels).
"""
import sys

sys.path.insert(0, "/opt/trn_rl_repo")

import numpy as np

import concourse.bass as bass
import concourse.tile as tile
from concourse import mybir, bacc
from concourse.bass_utils import run_bass_kernel_spmd

B, C = 2, 128
N_FULL = 48 * 96 * 96          # 442368
N_CORES = 8
Q_PER_B = 4                    # spatial quarters per batch
NS = N_FULL // Q_PER_B         # 110592 per core
F_CHUNK = 2048                 # spatial columns per chunk
N_CHUNKS = NS // F_CHUNK       # 54
SUB = 512                      # matmul free dim (one PSUM bank fp32)
N_SUB = F_CHUNK // SUB         # 4
EXP_BIAS = -8.0
STT_LAG = 2                    # chunks of lag between exp and its stt emission

f32 = mybir.dt.float32
f16 = mybir.dt.float16

_CACHED = {}


def _build(bench_reps=None, variant="full"):
    """bench_reps=None -> production straight-line kernel.
    bench_reps=R -> same body wrapped in a For_i(R) repeat loop (for HW
    timing via wall-clock deltas between two R values).
    variant: ablation selector ("full", "nostt", "nomm", "noexp", "dmaonly")."""
    nc = bacc.Bacc("TRN2", target_bir_lowering=False, debug=False)

    feat_dram = nc.dram_tensor("feat16", [C, NS], f16, kind="ExternalInput")
    mrows_dram = nc.dram_tensor("mrows", [2, NS], f16, kind="ExternalInput")
    whrep_dram = nc.dram_tensor("whrep", [C, 128], f16, kind="ExternalInput")
    wlrep_dram = nc.dram_tensor("wlrep", [C, 128], f16, kind="ExternalInput")
    out_dram = nc.dram_tensor("out_vs", [C, 2], f32, kind="ExternalOutput")

    with tile.TileContext(nc) as tc:
        with (
            tc.tile_pool(name="weights", bufs=1) as wpool,
            tc.tile_pool(name="feat", bufs=8) as apool,
            tc.tile_pool(name="mask", bufs=8) as mpool,
            tc.tile_pool(name="prob", bufs=6) as ppool,
            tc.tile_pool(name="junk", bufs=3) as jpool,
            tc.tile_pool(name="accs", bufs=1) as accpool,
            tc.tile_pool(name="psum", bufs=2, space="PSUM") as psum,
        ):
            whrep = wpool.tile([C, 128], f16)
            wlrep = wpool.tile([C, 128], f16)
            nc.sync.dma_start(whrep[:], whrep_dram.ap())
            nc.sync.dma_start(wlrep[:], wlrep_dram.ap())
            ones2 = wpool.tile([2, 128], f16)
            nc.vector.memset(ones2[:], 1.0)
            bias_t = wpool.tile([C, 1], f32)
            nc.vector.memset(bias_t[:], EXP_BIAS)

            v_accs = accpool.tile([C, N_CHUNKS], f32)
            s_accs = accpool.tile([C, N_CHUNKS], f32)
            if variant in ("dmaonly", "nostt", "noexp"):
                nc.vector.memset(v_accs[:], 1.0)
                nc.vector.memset(s_accs[:], 1.0)

            def emit_chunk(ci):
                fh = apool.tile([C, F_CHUNK], f16, tag="fh")
                nc.sync.dma_start(
                    fh[:], feat_dram.ap()[:, ci * F_CHUNK:(ci + 1) * F_CHUNK])
                mrows = mpool.tile([2, F_CHUNK], f16, tag="mrows")
                nc.sync.dma_start(
                    mrows[:], mrows_dram.ap()[:, ci * F_CHUNK:(ci + 1) * F_CHUNK])
                if variant == "dmaonly":
                    return None

                x = psum.tile([C, F_CHUNK], f32, tag="x")
                if variant != "nomm":
                    # grouped same-weight runs to minimize weight switching
                    for si in range(N_SUB):
                        nc.tensor.matmul(x[:, si * SUB:(si + 1) * SUB], whrep[:],
                                         fh[:, si * SUB:(si + 1) * SUB],
                                         start=True, stop=False)
                    for si in range(N_SUB):
                        nc.tensor.matmul(x[:, si * SUB:(si + 1) * SUB], wlrep[:],
                                         fh[:, si * SUB:(si + 1) * SUB],
                                         start=False, stop=False)
                    for si in range(N_SUB):
                        nc.tensor.matmul(x[:, si * SUB:(si + 1) * SUB], ones2[:],
                                         mrows[:, si * SUB:(si + 1) * SUB],
                                         start=False, stop=True)

                pb = ppool.tile([C, F_CHUNK], f32, tag="pb")
                if variant != "noexp":
                    nc.scalar.activation(
                        pb[:], x[:], mybir.ActivationFunctionType.Exp,
                        bias=bias_t[:], scale=1.0,
                        accum_out=s_accs[:, ci:ci + 1],
                    )
                return fh, pb

            def emit_stt(ci, fh, pb):
                if variant in ("nostt", "dmaonly"):
                    return
                junk = jpool.tile([C, F_CHUNK], f32, tag="junk")
                nc.vector.scalar_tensor_tensor(
                    junk[:], fh[:], 1.0, pb[:],
                    op0=mybir.AluOpType.mult, op1=mybir.AluOpType.mult,
                    accum_out=v_accs[:, ci:ci + 1],
                )

            def emit_all():
                # the stt for chunk ci is emitted STT_LAG chunks later so DVE
                # never sits on the critical path (its inputs are long-ready)
                pending = []
                for ci in range(N_CHUNKS):
                    r = emit_chunk(ci)
                    if r is not None:
                        pending.append((ci, *r))
                    while len(pending) > STT_LAG:
                        emit_stt(*pending.pop(0))
                while pending:
                    emit_stt(*pending.pop(0))

            if bench_reps is None:
                emit_all()
            else:
                with tc.For_i(0, bench_reps, 1,
                              hint_engines=(mybir.EngineType.PE,)):
                    emit_all()

            out_sb = accpool.tile([C, 2], f32)
            nc.vector.reduce_sum(out_sb[:, 0:1], v_accs[:], axis=mybir.AxisListType.X)
            nc.vector.reduce_sum(out_sb[:, 1:2], s_accs[:], axis=mybir.AxisListType.X)
            nc.sync.dma_start(out_dram.ap(), out_sb[:])

    nc.compile()
    return nc


def _get_nc(bench_reps=None, variant="full"):
    key = (bench_reps, variant)
    if key not in _CACHED:
        _CACHED[key] = _build(bench_reps, variant)
    return _CACHED[key]


def make_in_maps(feat, mask, w_attn):
    feat2 = feat.reshape(B, C, N_FULL)
    mask2 = 2.0 * np.clip(mask.reshape(B, N_FULL).astype(np.float64), 0.0, 1.0)
    w32 = w_attn.astype(np.float32)
    wh = w32.astype(np.float16)
    wl = (w32.astype(np.float64) - wh.astype(np.float64)).astype(np.float16)
    whrep = np.ascontiguousarray(np.tile(wh[:, None], (1, 128)))
    wlrep = np.ascontiguousarray(np.tile(wl[:, None], (1, 128)))
    in_maps = []
    for core in range(N_CORES):
        b, q = divmod(core, Q_PER_B)
        fh = feat2[b, :, q * NS:(q + 1) * NS].astype(np.float16)
        m2 = mask2[b, q * NS:(q + 1) * NS]
        mh = m2.astype(np.float16)
        ml = (m2 - mh.astype(np.float64)).astype(np.float16)
        in_maps.append({
            "feat16": np.ascontiguousarray(fh),
            "mrows": np.ascontiguousarray(np.stack([mh, ml])),
            "whrep": whrep,
            "wlrep": wlrep,
        })
    return in_maps


def combine(results):
    out = np.zeros((B, C), dtype=np.float32)
    for b in range(B):
        v = np.zeros(C, dtype=np.float64)
        s = 0.0
        for q in range(Q_PER_B):
            r = results[b * Q_PER_B + q]["out_vs"]
            v += r[:, 0].astype(np.float64)
            s += float(r[0, 1])
        out[b] = (v / s).astype(np.float32)
    return out


def run_on_cores(feat, mask, w_attn, bench_reps=None):
    nc = _get_nc(bench_reps)
    in_maps = make_in_maps(np.asarray(feat), np.asarray(mask), np.asarray(w_attn))
    res = run_bass_kernel_spmd(nc, in_maps, core_ids=list(range(N_CORES)))
    return res


def kernel(feat, mask, w_attn):
    res = run_on_cores(feat, mask, w_attn)
    return combine(res.results)


# revision 13
# speedup vs baseline: 1.0026x; 1.0026x over previous
"""AttnPool3D Trainium2 kernel.

Reference computation (B=2, C=128, D=48, H=96, W=96, N = D*H*W = 442368):
    logits = einsum('bcdhw,c->bdhw', feat, w_attn) + 2.0*clip(mask, 0, 1)
    w = softmax(logits.reshape(B, -1), axis=-1)
    out = einsum('bcn,bn->bc', feat.reshape(B, C, -1), w)

Sharding: 8 cores = (batch b in 0..1) x (spatial quarter q in 0..3).
Each core processes feat[b, :, q*Ns:(q+1)*Ns] (Ns = 110592) in ONE pass.

Numerics: feat is shipped as fp16 (fh). Per-element fp16 rounding noise is
independent of the softmax weights, so it averages out in the final weighted
sums (measured ~1e-5 L2 rel err end-to-end). The w vector, by contrast, is
fp16-split (w = wh + wl) because its rounding error correlates with the
pass-2 data and does NOT average out (dropping wl costs 2e-4 rel err).
The mask is fp16-split the same way (mh + ml rows, one K=2 matmul).
Softmax runs without a max pass: logits are bounded (~N(0,1.3)+[0,2]); a
constant bias -8 in the exp prevents overflow and cancels in v/s.

Per chunk of 2048 spatial columns (54 chunks):
    - DMA fh chunk [128, 2048] fp16 (C on partitions), mask rows [2, 2048]
    - PE (per 512-sub-chunk, accumulating into PSUM [128, 512]):
        X  = wh_rep^T @ fh       (broadcasts logits to all 128 partitions)
        X += wl_rep^T @ fh
        X += ones2^T @ [mh; ml]
    - ACT: Pb = exp(X - 8), accum_out -> s_chunk [128, 1]
    - DVE: stt junk = fh * Pb (fp32 ALU), accum_out -> v_chunk [128, 1]
Host combines: out[b, c] = sum_q v / sum_q s (fp64; the -8 bias cancels).
"""
import sys

sys.path.insert(0, "/opt/trn_rl_repo")

import numpy as np

import concourse.bass as bass
import concourse.tile as tile
from concourse import mybir, bacc
from concourse.bass_utils import run_bass_kernel_spmd

B, C = 2, 128
N_FULL = 48 * 96 * 96          # 442368
N_CORES = 8
Q_PER_B = 4                    # spatial quarters per batch
NS = N_FULL // Q_PER_B         # 110592 per core
F_CHUNK = 2048                 # spatial columns per chunk
N_CHUNKS = NS // F_CHUNK       # 54
SUB = 512                      # matmul free dim (one PSUM bank fp32)
N_SUB = F_CHUNK // SUB         # 4
EXP_BIAS = -8.0
STT_LAG = 2                    # chunks of lag between exp and its stt emission

f32 = mybir.dt.float32
f16 = mybir.dt.float16

_CACHED = {}


def _build(bench_reps=None, variant="full"):
    """bench_reps=None -> production straight-line kernel.
    bench_reps=R -> same body wrapped in a For_i(R) repeat loop (for HW
    timing via wall-clock deltas between two R values).
    variant: ablation selector ("full", "nostt", "nomm", "noexp", "dmaonly")."""
    nc = bacc.Bacc("TRN2", target_bir_lowering=False, debug=False)

    feat_dram = nc.dram_tensor("feat16", [C, NS], f16, kind="ExternalInput")
    mrows_dram = nc.dram_tensor("mrows", [2, NS], f16, kind="ExternalInput")
    whrep_dram = nc.dram_tensor("whrep", [C, 128], f16, kind="ExternalInput")
    wlrep_dram = nc.dram_tensor("wlrep", [C, 128], f16, kind="ExternalInput")
    out_dram = nc.dram_tensor("out_vs", [C, 2], f32, kind="ExternalOutput")

    with tile.TileContext(nc) as tc:
        with (
            tc.tile_pool(name="weights", bufs=1) as wpool,
            tc.tile_pool(name="feat", bufs=8) as apool,
            tc.tile_pool(name="mask", bufs=8) as mpool,
            tc.tile_pool(name="prob", bufs=(16 if variant == "banks" else 6)) as ppool,
            tc.tile_pool(name="junk", bufs=(6 if variant == "banks" else 3)) as jpool,
            tc.tile_pool(name="accs", bufs=1) as accpool,
            tc.tile_pool(name="psum", bufs=(8 if variant == "banks" else 2), space="PSUM") as psum,
        ):
            whrep = wpool.tile([C, 128], f16)
            wlrep = wpool.tile([C, 128], f16)
            nc.sync.dma_start(whrep[:], whrep_dram.ap())
            nc.sync.dma_start(wlrep[:], wlrep_dram.ap())
            ones2 = wpool.tile([2, 128], f16)
            nc.vector.memset(ones2[:], 1.0)
            bias_t = wpool.tile([C, 1], f32)
            nc.vector.memset(bias_t[:], EXP_BIAS)

            acc_w = N_CHUNKS * N_SUB if variant == "banks" else N_CHUNKS
            v_accs = accpool.tile([C, acc_w], f32)
            s_accs = accpool.tile([C, acc_w], f32)
            if variant in ("dmaonly", "nostt", "noexp"):
                nc.vector.memset(v_accs[:], 1.0)
                nc.vector.memset(s_accs[:], 1.0)

            def emit_chunk_banks(ci):
                # bank-granular: psum tiles are one bank [C, SUB]; exp/stt per
                # sub-chunk so ACT/DVE overlap PE inside a chunk
                fh = apool.tile([C, F_CHUNK], f16, tag="fh")
                nc.sync.dma_start(
                    fh[:], feat_dram.ap()[:, ci * F_CHUNK:(ci + 1) * F_CHUNK])
                mrows = mpool.tile([2, F_CHUNK], f16, tag="mrows")
                nc.sync.dma_start(
                    mrows[:], mrows_dram.ap()[:, ci * F_CHUNK:(ci + 1) * F_CHUNK])
                pbs = []
                for si in range(N_SUB):
                    sl = slice(si * SUB, (si + 1) * SUB)
                    xb = psum.tile([C, SUB], f32, tag="xb")
                    nc.tensor.matmul(xb[:], whrep[:], fh[:, sl], start=True, stop=False)
                    nc.tensor.matmul(xb[:], wlrep[:], fh[:, sl], start=False, stop=False)
                    nc.tensor.matmul(xb[:], ones2[:], mrows[:, sl], start=False, stop=True)
                    pbb = ppool.tile([C, SUB], f32, tag="pbb")
                    nc.scalar.activation(
                        pbb[:], xb[:], mybir.ActivationFunctionType.Exp,
                        bias=bias_t[:], scale=1.0,
                        accum_out=s_accs[:, ci * N_SUB + si:ci * N_SUB + si + 1],
                    )
                    pbs.append(pbb)
                return fh, pbs

            def emit_stt_banks(ci, fh, pbs):
                for si in range(N_SUB):
                    junk = jpool.tile([C, SUB], f32, tag="junkb")
                    nc.vector.scalar_tensor_tensor(
                        junk[:], fh[:, si * SUB:(si + 1) * SUB], 1.0, pbs[si][:],
                        op0=mybir.AluOpType.mult, op1=mybir.AluOpType.mult,
                        accum_out=v_accs[:, ci * N_SUB + si:ci * N_SUB + si + 1],
                    )

            def emit_chunk(ci):
                fh = apool.tile([C, F_CHUNK], f16, tag="fh")
                nc.sync.dma_start(
                    fh[:], feat_dram.ap()[:, ci * F_CHUNK:(ci + 1) * F_CHUNK])
                mrows = mpool.tile([2, F_CHUNK], f16, tag="mrows")
                nc.sync.dma_start(
                    mrows[:], mrows_dram.ap()[:, ci * F_CHUNK:(ci + 1) * F_CHUNK])
                if variant == "dmaonly":
                    return None

                x = psum.tile([C, F_CHUNK], f32, tag="x")
                if variant != "nomm":
                    # grouped same-weight runs to minimize weight switching
                    for si in range(N_SUB):
                        nc.tensor.matmul(x[:, si * SUB:(si + 1) * SUB], whrep[:],
                                         fh[:, si * SUB:(si + 1) * SUB],
                                         start=True, stop=False)
                    for si in range(N_SUB):
                        nc.tensor.matmul(x[:, si * SUB:(si + 1) * SUB], wlrep[:],
                                         fh[:, si * SUB:(si + 1) * SUB],
                                         start=False, stop=False)
                    for si in range(N_SUB):
                        nc.tensor.matmul(x[:, si * SUB:(si + 1) * SUB], ones2[:],
                                         mrows[:, si * SUB:(si + 1) * SUB],
                                         start=False, stop=True)

                pb = ppool.tile([C, F_CHUNK], f32, tag="pb")
                if variant != "noexp":
                    nc.scalar.activation(
                        pb[:], x[:], mybir.ActivationFunctionType.Exp,
                        bias=bias_t[:], scale=1.0,
                        accum_out=s_accs[:, ci:ci + 1],
                    )
                return fh, pb

            def emit_stt(ci, fh, pb):
                if variant in ("nostt", "dmaonly"):
                    return
                junk = jpool.tile([C, F_CHUNK], f32, tag="junk")
                nc.vector.scalar_tensor_tensor(
                    junk[:], fh[:], 1.0, pb[:],
                    op0=mybir.AluOpType.mult, op1=mybir.AluOpType.mult,
                    accum_out=v_accs[:, ci:ci + 1],
                )

            def emit_all():
                # the stt for chunk ci is emitted STT_LAG chunks later so DVE
                # never sits on the critical path (its inputs are long-ready)
                pending = []
                ec = emit_chunk_banks if variant == "banks" else emit_chunk
                es = emit_stt_banks if variant == "banks" else emit_stt
                for ci in range(N_CHUNKS):
                    r = ec(ci)
                    if r is not None:
                        pending.append((ci, *r))
                    while len(pending) > STT_LAG:
                        es(*pending.pop(0))
                while pending:
                    es(*pending.pop(0))

            if bench_reps is None:
                emit_all()
            else:
                with tc.For_i(0, bench_reps, 1,
                              hint_engines=(mybir.EngineType.PE,)):
                    emit_all()

            out_sb = accpool.tile([C, 2], f32)
            nc.vector.reduce_sum(out_sb[:, 0:1], v_accs[:], axis=mybir.AxisListType.X)
            nc.vector.reduce_sum(out_sb[:, 1:2], s_accs[:], axis=mybir.AxisListType.X)
            nc.sync.dma_start(out_dram.ap(), out_sb[:])

    nc.compile()
    return nc


def _get_nc(bench_reps=None, variant="full"):
    key = (bench_reps, variant)
    if key not in _CACHED:
        _CACHED[key] = _build(bench_reps, variant)
    return _CACHED[key]


def make_in_maps(feat, mask, w_attn):
    feat2 = feat.reshape(B, C, N_FULL)
    mask2 = 2.0 * np.clip(mask.reshape(B, N_FULL).astype(np.float64), 0.0, 1.0)
    w32 = w_attn.astype(np.float32)
    wh = w32.astype(np.float16)
    wl = (w32.astype(np.float64) - wh.astype(np.float64)).astype(np.float16)
    whrep = np.ascontiguousarray(np.tile(wh[:, None], (1, 128)))
    wlrep = np.ascontiguousarray(np.tile(wl[:, None], (1, 128)))
    in_maps = []
    for core in range(N_CORES):
        b, q = divmod(core, Q_PER_B)
        fh = feat2[b, :, q * NS:(q + 1) * NS].astype(np.float16)
        m2 = mask2[b, q * NS:(q + 1) * NS]
        mh = m2.astype(np.float16)
        ml = (m2 - mh.astype(np.float64)).astype(np.float16)
        in_maps.append({
            "feat16": np.ascontiguousarray(fh),
            "mrows": np.ascontiguousarray(np.stack([mh, ml])),
            "whrep": whrep,
            "wlrep": wlrep,
        })
    return in_maps


def combine(results):
    out = np.zeros((B, C), dtype=np.float32)
    for b in range(B):
        v = np.zeros(C, dtype=np.float64)
        s = 0.0
        for q in range(Q_PER_B):
            r = results[b * Q_PER_B + q]["out_vs"]
            v += r[:, 0].astype(np.float64)
            s += float(r[0, 1])
        out[b] = (v / s).astype(np.float32)
    return out


def run_on_cores(feat, mask, w_attn, bench_reps=None):
    nc = _get_nc(bench_reps)
    in_maps = make_in_maps(np.asarray(feat), np.asarray(mask), np.asarray(w_attn))
    res = run_bass_kernel_spmd(nc, in_maps, core_ids=list(range(N_CORES)))
    return res


def kernel(feat, mask, w_attn):
    res = run_on_cores(feat, mask, w_attn)
    return combine(res.results)
